# revision 1
# baseline (speedup 1.0000x reference)
"""BiMPM matching-layer kernel for Trainium2 (8 NeuronCores, pure data parallel).

Computes, per batch b and direction d (fw/bw, H=128 halves of the 256-dim inputs):
  m1: full matching vs last/first q timestep
  m2: max-pooling matching (max over q of per-perspective cosine)
  m3: mean-attentive matching (cosine-matrix weighted mean, scale-invariant form)
  m4: max-attentive matching (argmax over q of cosine matrix; gathers batch-0 q rows,
      faithful to the reference's flattened-index behavior)
Output (B, S1, 8*L) with L=20 perspectives.

Sharding: batch dim 64 -> 8 batches per core. Inputs are laid out host-side
(transposed copies, squared weights) so the device only does matmuls, scaling,
and the max reductions.

m2 pipeline (the hot path, all fp16): v2wall_l = v2 * w2_l[h] (TSP 4x mode),
v2ws = v2wall_l * rs2_l[q] (TT 2x mode, two perspectives per op; the rs2 rows
are replicated across partitions by a DMA broadcast through a DRAM bounce), so
the PE matmul emits scaled cosine numerators directly; ACT drains each 2-bank
PSUM pair into the fp16 stage, and a log2 tree of 2x-mode TT-max ops reduces
over q, software-pipelined one (b,d) pair late so it overlaps the ACT drains.
"""
import numpy as np

B, S, H, L = 64, 256, 128, 20
NCORES = 8
BPC = B // NCORES
EPS = 1e-8

_cache = {}


def _build_bass():
    from contextlib import ExitStack

    import concourse.bass as bass
    import concourse.tile as tile
    from concourse import mybir

    f32 = mybir.dt.float32
    bf16 = mybir.dt.bfloat16
    f16 = mybir.dt.float16
    AF = mybir.ActivationFunctionType
    OP = mybir.AluOpType

    nc = bass.Bass()

    # DRAM I/O (per core)
    pT = nc.dram_tensor("pT", [BPC, 2, H, S], f32, kind="ExternalInput")
    pTb = nc.dram_tensor("pTb", [BPC, 2, H, S], f16, kind="ExternalInput")
    qT = nc.dram_tensor("qT", [BPC, 2, H, S], f32, kind="ExternalInput")
    qTb = nc.dram_tensor("qTb", [BPC, 2, H, S], f16, kind="ExternalInput")
    qn = nc.dram_tensor("qn", [BPC, 2, S, H], f32, kind="ExternalInput")
    q0nb = nc.dram_tensor("q0nb", [2, S, H], f16, kind="ExternalInput")
    w2T = nc.dram_tensor("w2T", [2, H, 81], f32, kind="ExternalInput")
    iota2 = nc.dram_tensor("iota2", [H, 2], f32, kind="ExternalInput")
    onesr = nc.dram_tensor("onesr", [1, H], f32, kind="ExternalInput")
    ident = nc.dram_tensor("ident", [H, H], f32, kind="ExternalInput")
    out = nc.dram_tensor("out", [BPC, S, 8 * L], f32, kind="ExternalOutput")
    # DRAM bounces for row broadcasts (per-pair slots: no WAR hazards)
    r2scr = nc.dram_tensor("r2scr", [BPC, 2, 1, L * S], f16, kind="Internal")
    idxscr = nc.dram_tensor("idxscr", [BPC, 2, 1, S], f16, kind="Internal")

    with tile.TileContext(nc) as tc, ExitStack() as ctx:
        cpool = ctx.enter_context(tc.tile_pool(name="consts", bufs=1))
        inp = ctx.enter_context(tc.tile_pool(name="inp", bufs=4))
        mid = ctx.enter_context(tc.tile_pool(name="mid", bufs=2))
        m2p = ctx.enter_context(tc.tile_pool(name="m2p", bufs=8))
        stg = ctx.enter_context(tc.tile_pool(name="stg", bufs=2))
        repp = ctx.enter_context(tc.tile_pool(name="repp", bufs=2))
        treep = ctx.enter_context(tc.tile_pool(name="treep", bufs=2))
        outp = ctx.enter_context(tc.tile_pool(name="outp", bufs=3))
        # PSUM: 8 banks. psNum holds two perspectives (2 banks) x2 bufs; the
        # other pools share banks via same-tag reuse (psT->psQ, psG->psGT,
        # psWork->psNorm).
        psSmall_p = ctx.enter_context(tc.tile_pool(name="psSmall", bufs=1, space="PSUM"))
        psNorm_p = ctx.enter_context(tc.tile_pool(name="psNorm", bufs=1, space="PSUM"))
        psQ_p = ctx.enter_context(tc.tile_pool(name="psQ", bufs=1, space="PSUM"))
        psGT_p = ctx.enter_context(tc.tile_pool(name="psGT", bufs=1, space="PSUM"))
        psNum_p = ctx.enter_context(tc.tile_pool(name="psNum", bufs=2, space="PSUM"))

        # constants
        w2_sb = cpool.tile([H, 2, 81], f32)
        nc.sync.dma_start(out=w2_sb, in_=w2T[:].rearrange("d h c -> h d c"))
        q0cb = cpool.tile([H, 2, 2, H], f16)
        nc.sync.dma_start(out=q0cb, in_=q0nb[:].rearrange("d (c q) h -> q d c h", c=2))
        iota_sb = cpool.tile([H, 2], f32)
        nc.sync.dma_start(out=iota_sb, in_=iota2[:])
        onesr_sb = cpool.tile([1, H], f32)
        nc.sync.dma_start(out=onesr_sb, in_=onesr[:])
        ident_sb = cpool.tile([H, H], f32)
        nc.sync.dma_start(out=ident_sb, in_=ident[:])
        # Touch every const once on DVE so const-DMA waits are absorbed here and
        # later DVE instructions (limited sync-wait slots) don't re-carry them.
        warm = cpool.tile([H, 8], f32)
        nc.vector.tensor_copy(warm[:, 0:1], w2_sb[:, 0, 0:1])
        nc.vector.tensor_copy(warm[:, 1:2], w2_sb[:, 1, 0:1])
        nc.vector.tensor_copy(warm[:, 2:3].bitcast(f16)[:, 0:1], q0cb[:, 0, 0, 0:1])
        nc.vector.tensor_copy(warm[:, 3:4], iota_sb[:, 0:1])
        nc.vector.tensor_copy(warm[0:1, 4:5], onesr_sb[0:1, 0:1])
        nc.vector.tensor_copy(warm[:, 6:7], ident_sb[:, 0:1])
        # PE const absorbs (chained): PE instructions may carry only one
        # (possibly multi-queue) DMA wait, so consume each const's DMA sem once.
        # ldweights has no output tile, so it adds no PSUM slot/release deps.
        # (dtype is irrelevant -- all real matmuls are self-loading.)
        ab_w2 = nc.tensor.ldweights(w2_sb[:, 0, 0:1].bitcast(bf16))
        ab_q0 = nc.tensor.ldweights(q0cb[:, 0, 0, 0:1])
        ab_id = nc.tensor.ldweights(ident_sb[:, 0:1].bitcast(bf16))
        bass._add_dep_helper(ab_q0.ins, ab_w2.ins, sync=False, reason="const absorb chain")
        bass._add_dep_helper(ab_id.ins, ab_q0.ins, sync=False, reason="const absorb chain")

        def dep(from_inst, to_inst, why="absorb order"):
            bass._add_dep_helper(from_inst.ins, to_inst.ins, sync=False, reason=why)

        def emit_m2_tail_rest(state, tr64):
            """Tree levels 3+ and m2 finals (+ output store after d=1);
            levels 1-2 run chunk-wise inside the owning pair's l-loop."""
            stage, rsa_t, outts_t, b_t, d_t = state
            cur = tr64
            w = 32
            while w >= 1:
                nxt = treep.tile([H, 2, L, w], f16, name=f"tr{w}")
                nc.vector.tensor_tensor(
                    nxt, cur[:, :, :, 0:w], cur[:, :, :, w : 2 * w], op=OP.max
                )
                cur = nxt
                w //= 2
            for c in range(2):
                nc.vector.tensor_tensor(
                    outts_t[c][:, 40 + d_t * 20 : 40 + d_t * 20 + 20],
                    cur[:, c, :, 0],
                    rsa_t[:, 81 * c + 20 : 81 * c + 40],
                    op=OP.mult,
                )
            if d_t == 1:
                nc.sync.dma_start(out=out[b_t, 0:H, :], in_=outts_t[0])
                nc.sync.dma_start(out=out[b_t, H:S, :], in_=outts_t[1])

        pend = None
        for b in range(BPC):
            outt0 = outp.tile([H, 8 * L], f32, name="outt0")
            outt1 = outp.tile([H, 8 * L], f32, name="outt1")
            outts = (outt0, outt1)
            for d in range(2):
                w2d = w2_sb[:, d, :]
                # ---- loads ----
                v1Tf = inp.tile([H, S], f32, name="v1Tf")
                nc.sync.dma_start(out=v1Tf, in_=pT[b, d])
                v1Tb = inp.tile([H, S], f16, name="v1Tb")
                nc.sync.dma_start(out=v1Tb, in_=pTb[b, d])
                v2Tf = inp.tile([H, S], f32, name="v2Tf")
                nc.sync.dma_start(out=v2Tf, in_=qT[b, d])
                v2Tb = inp.tile([H, S], f16, name="v2Tb")
                nc.sync.dma_start(out=v2Tb, in_=qTb[b, d])
                v2nat = inp.tile([H, 2, H], f32, name="v2nat")
                nc.sync.dma_start(out=v2nat, in_=qn[b, d].rearrange("(c q) h -> q c h", c=2))
                first = b == 0 and d == 0
                # per-pair DVE absorbs of big-load DMA sems
                dve_scr = mid.tile([H, 3], f32, name="dve_scr")
                abs_v2nat = nc.vector.tensor_copy(dve_scr[:, 0:1], v2nat[:, 0, 0:1])
                abs_v1Tf_dve = nc.vector.tensor_copy(dve_scr[:, 1:2], v1Tf[:, 0:1])
                abs_v2Tb_dve = nc.vector.tensor_copy(dve_scr[:, 2:3].bitcast(f16)[:, 0:1], v2Tb[:, 0:1])
                # per-pair PE absorbs (chained) for the fp32/bf16 big loads PE reads
                psSmall = psSmall_p.tile([H, 512], f32, name="psSmall")
                ab1 = nc.tensor.ldweights(v1Tf[:, 0:1].bitcast(bf16))
                ab2 = nc.tensor.ldweights(v2Tf[:, 0:1].bitcast(bf16))
                ab3 = nc.tensor.ldweights(v1Tb[:, 0:1])
                dep(ab2, ab1, "pe absorb chain")
                dep(ab3, ab2, "pe absorb chain")
                if first:
                    dep(ab1, ab_id, "consts before pair absorbs")

                # ---- squares (Pool; ACT is near-saturated) & norm matmuls ----
                v1sq = mid.tile([H, S], f32, name="v1sq")
                nc.gpsimd.tensor_tensor(v1sq, v1Tf, v1Tf, op=OP.mult)
                v2sq = mid.tile([H, S], f32, name="v2sq")
                nc.gpsimd.tensor_tensor(v2sq, v2Tf, v2Tf, op=OP.mult)

                psNorm = psNorm_p.tile([H, 512], f32, name="psNorm")
                # [0:81 n1-c0 | 81:162 n1-c1 | 162:163 n2ones-c0 | 163:164 n2ones-c1]
                # (of the n2 norms only the plain-ones column is ever used:
                # m2's per-perspective rs2 comes from the q-layout psQ path)
                norm_mms = [
                    nc.tensor.matmul(psNorm[:, 0:81], v1sq[:, 0:H], w2d, start=True, stop=True),
                    nc.tensor.matmul(psNorm[:, 81:162], v1sq[:, H:S], w2d, start=True, stop=True),
                    nc.tensor.matmul(psNorm[:, 162:163], v2sq[:, 0:H], w2d[:, 80:81], start=True, stop=True),
                    nc.tensor.matmul(psNorm[:, 163:164], v2sq[:, H:S], w2d[:, 80:81], start=True, stop=True),
                ]

                psQ = psQ_p.tile([H, 512], f32, name="psQ")
                # n2bT (20, 256) at [0:20, 0:256]
                norm_mms.append(
                    nc.tensor.matmul(psQ[0:20, 0:256], w2d[:, 20:40], v2sq, start=True, stop=True)
                )
                if first:
                    for mm in norm_mms:
                        dep(mm, ab_id, "PE consts absorbed before first norm mms")

                # rsqrt of all norms: reciprocal (DVE) then sqrt (ACT)
                # rsa layout: [0:162 n1 (c0|c1) | 162:164 n2-ones (c0|c1)]
                rsa = mid.tile([H, 164], f32, name="rsa")
                nc.vector.reciprocal(rsa, psNorm[:, 0:164])
                nc.scalar.sqrt(rsa, rsa)
                r2bT = mid.tile([20, 256], f32, name="r2bT")
                nc.vector.reciprocal(r2bT, psQ[0:20, 0:256])
                r2bTb = mid.tile([20, 256], f16, name="r2bTb")
                nc.scalar.sqrt(r2bTb, r2bT)
                # broadcast the 20 rs2 rows to all 128 partitions via a DRAM bounce
                nc.sync.dma_start(
                    out=r2scr[b, d].rearrange("o (l s) -> (o l) s", l=L), in_=r2bTb[:]
                )
                repb = repp.tile([H, L, S], f16, name="repb")
                nc.sync.dma_start(
                    out=repb[:].rearrange("h l s -> h (l s)"),
                    in_=r2scr[b, d].to_broadcast((H, L * S)),
                )

                # eps * ||v1|| per chunk (for the d-sign)
                nv1e0 = mid.tile([H, 1], f32, name="nv1e0")
                nc.scalar.sqrt(nv1e0, psNorm[:, 80:81])
                nc.scalar.mul(nv1e0, nv1e0, EPS)
                nv1e1 = mid.tile([H, 1], f32, name="nv1e1")
                nc.scalar.sqrt(nv1e1, psNorm[:, 161:162])
                nc.scalar.mul(nv1e1, nv1e1, EPS)

                # psSmall layout:
                # [c*100+ : num1 0:20 | num3 20:40 | num4 40:60 | n3c 60:80 | n4d 80:100]
                # [200:201 Gr-c0 | 201:202 Gr-c1 | 202:222 n2a(1row) | 222:242 rep-rs2a | 242:498 idxT(1row)]

                # ---- m1 numerators (finals deferred to cover rep latency);
                # the n2a chain avoids ACT so it never queues behind the
                # previous pair's stage drains ----
                tcol = v2Tf[:, 255:256] if d == 0 else v2Tf[:, 0:1]
                sqt = mid.tile([H, 1], f32, name="sqt")
                nc.gpsimd.tensor_tensor(sqt, tcol, tcol, op=OP.mult)
                rhs1 = mid.tile([H, 20], f32, name="rhs1")
                nc.vector.tensor_scalar_mul(rhs1, w2d[:, 0:20], tcol)
                n2a_mm = nc.tensor.matmul(psSmall[0:1, 202:222], sqt, w2d[:, 0:20], start=True, stop=True)
                if first:
                    dep(n2a_mm, ab_id, "consts before n2a")
                n2a_sb = mid.tile([1, 20], f32, name="n2a_sb")
                nc.vector.tensor_copy(n2a_sb, psSmall[0:1, 202:222])
                n2a_rep_mm = nc.tensor.matmul(psSmall[:, 222:242], onesr_sb, n2a_sb, start=True, stop=True)
                num1_mms = [
                    nc.tensor.matmul(psSmall[:, 0:20], v1Tf[:, 0:H], rhs1, start=True, stop=True),
                    nc.tensor.matmul(psSmall[:, 100:120], v1Tf[:, H:S], rhs1, start=True, stop=True),
                ]
                for mm in num1_mms:
                    dep(mm, ab1, "v1Tf absorbed before num1")

                # ---- GT (fp32) ----
                psGT = psGT_p.tile([H, 512], f32, name="psGT")
                gt_mms = [
                    nc.tensor.matmul(psGT[:, 0:256], v2Tf[:, 0:H], v1Tf, start=True, stop=True),
                    nc.tensor.matmul(psGT[:, 256:512], v2Tf[:, H:S], v1Tf, start=True, stop=True),
                ]
                for mm in gt_mms:
                    dep(mm, ab2, "PE absorbs dma sems before GT")
                GT_sb = mid.tile([H, 2, S], f32, name="GT_sb")
                nc.scalar.copy(GT_sb[:].rearrange("h c q -> h (c q)"), psGT[:, 0:512])

                # v2r = v2 * (1/||v2||) rows  (q on partitions); on Pool
                v2r = mid.tile([H, 2, H], f32, name="v2r")
                nc.gpsimd.tensor_scalar_mul(v2r[:, 0, :], v2nat[:, 0, :], rsa[:, 162:163])
                nc.gpsimd.tensor_scalar_mul(v2r[:, 1, :], v2nat[:, 1, :], rsa[:, 163:164])

                psWork = psNorm_p.tile([H, 512], f32, name="psNorm")  # reuse bank
                # GWT at [0:256], att4T at [256:512]
                nc.tensor.matmul(psWork[:, 0:256], v2r[:, 0, :], GT_sb[:, 0, :], start=True, stop=False)
                nc.tensor.matmul(psWork[:, 0:256], v2r[:, 1, :], GT_sb[:, 1, :], start=False, stop=True)

                # previous pair's tree levels 2+ (its level 1 already ran
                # chunk-wise inside that pair's l-loop)
                if pend is not None:
                    emit_m2_tail_rest(*pend)

                # ---- argmax over q of cos: G' = v1 . v2n (rs1[p] scale drops out) ----
                # transpose v2r (q-part, h) -> v2nT (h-part, q); then G' = v1^T v2nT
                psT = psQ_p.tile([H, 512], f32, name="psQ")  # reuse the psQ bank
                tr0 = nc.tensor.transpose(psT[:, 0:H], v2r[:, 0, :], ident_sb)
                tr1 = nc.tensor.transpose(psT[:, H : 2 * H], v2r[:, 1, :], ident_sb)
                if first:
                    dep(tr0, ab_id, "ident absorbed before transpose")
                    dep(tr1, ab_id, "ident absorbed before transpose")
                v2nT = mid.tile([H, S], f32, name="v2nT")
                nc.scalar.copy(v2nT, psT[:, 0:256])
                psG = psGT_p.tile([H, 512], f32, name="psGT")  # reuse bank
                g2_mms = [
                    nc.tensor.matmul(psG[:, 0:256], v1Tf[:, 0:H], v2nT, start=True, stop=True),
                    nc.tensor.matmul(psG[:, 256:512], v1Tf[:, H:S], v2nT, start=True, stop=True),
                ]
                for mm in g2_mms:
                    dep(mm, ab1, "v1Tf absorbed before G'")

                idxf = mid.tile([H, 2], f32, name="idxf")
                for c in range(2):
                    top8 = mid.tile([H, 8], f32, name="top8")
                    idx8 = mid.tile([H, 8], mybir.dt.uint32, name="idx8")
                    nc.vector.max_with_indices(top8, idx8, psG[:, 256 * c : 256 * c + 256])
                    nc.vector.tensor_copy(idxf[:, c : c + 1], idx8[:, 0:1])
                # transpose idxf columns -> row (1, 256) in psSmall[0:1, 242:498]
                for c in range(2):
                    tr = nc.tensor.transpose(
                        psSmall[0:1, 242 + c * H : 242 + c * H + H], idxf[:, c : c + 1], ident_sb
                    )
                    if first:
                        dep(tr, ab_id, "ident absorbed before transpose")
                # bf16 row of indices (ints <= 255: exact), broadcast via DRAM bounce
                idxT_sb = mid.tile([1, 256], f16, name="idxT_sb")
                nc.scalar.copy(idxT_sb, psSmall[0:1, 242:498])
                nc.sync.dma_start(out=idxscr[b, d], in_=idxT_sb[:])
                idxrepb = mid.tile([H, 256], f16, name="idxrepb")
                nc.sync.dma_start(out=idxrepb, in_=idxscr[b, d].to_broadcast((H, S)))

                # Gr sign + m3 numerators + m1 finals: independent work that
                # covers the idx-broadcast DMA latency
                for c in range(2):
                    nc.tensor.matmul(
                        psSmall[:, 200 + c : 201 + c],
                        GT_sb[:, 0, c * H : c * H + H],
                        rsa[:, 162:163],
                        start=True,
                        stop=False,
                    )
                    nc.tensor.matmul(
                        psSmall[:, 200 + c : 201 + c],
                        GT_sb[:, 1, c * H : c * H + H],
                        rsa[:, 163:164],
                        start=False,
                        stop=True,
                    )
                sgn0 = mid.tile([H, 1], f32, name="sgn0")
                nc.scalar.activation(sgn0, psSmall[:, 200:201], AF.Sign, bias=nv1e0, scale=1.0)
                sgn1 = mid.tile([H, 1], f32, name="sgn1")
                nc.scalar.activation(sgn1, psSmall[:, 201:202], AF.Sign, bias=nv1e1, scale=1.0)

                prod3 = mid.tile([H, S], f32, name="prod3")
                p3_tt = nc.vector.tensor_tensor(prod3, v1Tf, psWork[:, 0:256], op=OP.mult)
                dep(p3_tt, abs_v1Tf_dve, "v1Tf absorbed on DVE")
                sq3 = mid.tile([H, S], f32, name="sq3")
                nc.scalar.square(sq3, psWork[:, 0:256])
                for c in range(2):
                    base = 100 * c
                    sl = slice(c * H, c * H + H)
                    nc.tensor.matmul(psSmall[:, base + 20 : base + 40], prod3[:, sl], w2d[:, 40:60], start=True, stop=True)
                    nc.tensor.matmul(psSmall[:, base + 60 : base + 80], sq3[:, sl], w2d[:, 40:60], start=True, stop=True)

                rs2rep = mid.tile([H, 20], f32, name="rs2rep")
                nc.vector.reciprocal(rs2rep, psSmall[:, 222:242])
                nc.scalar.sqrt(rs2rep, rs2rep)
                # both c-chunks of num1*rs1 in one strided TT
                t1b = mid.tile([H, 2, 20], f32, name="t1b")
                nc.vector.tensor_tensor(
                    t1b,
                    psSmall[:, 0:200].rearrange("p (c x) -> p c x", c=2)[:, :, 0:20],
                    rsa[:, 0:162].rearrange("p (c x) -> p c x", c=2)[:, :, 0:20],
                    op=OP.mult,
                )
                for c in range(2):
                    nc.vector.tensor_tensor(
                        outts[c][:, d * 20 : d * 20 + 20], t1b[:, c, :], rs2rep, op=OP.mult
                    )

                # masks (idx broadcast has landed) -> att4 -> m4 numerators
                maskT0 = mid.tile([H, 256], f16, name="maskT0")
                nc.vector.tensor_scalar(maskT0, idxrepb, iota_sb[:, 0:1], None, op0=OP.is_equal)
                maskT1 = mid.tile([H, 256], f16, name="maskT1")
                nc.vector.tensor_scalar(maskT1, idxrepb, iota_sb[:, 1:2], None, op0=OP.is_equal)
                a4_mm0 = nc.tensor.matmul(psWork[:, 256:512], q0cb[:, d, 0, :], maskT0, start=True, stop=False)
                a4_mm1 = nc.tensor.matmul(psWork[:, 256:512], q0cb[:, d, 1, :], maskT1, start=False, stop=True)
                if first:
                    dep(a4_mm0, ab_id, "q0c absorbed before att4T")
                    dep(a4_mm1, ab_id, "q0c absorbed before att4T")

                prod4 = mid.tile([H, S], f32, name="prod4")
                p4_tt = nc.vector.tensor_tensor(prod4, v1Tf, psWork[:, 256:512], op=OP.mult)
                dep(p4_tt, abs_v1Tf_dve, "v1Tf absorbed on DVE")
                sq4 = mid.tile([H, S], f32, name="sq4")
                nc.scalar.square(sq4, psWork[:, 256:512])
                for c in range(2):
                    base = 100 * c
                    sl = slice(c * H, c * H + H)
                    nc.tensor.matmul(psSmall[:, base + 40 : base + 60], prod4[:, sl], w2d[:, 60:80], start=True, stop=True)
                    nc.tensor.matmul(psSmall[:, base + 80 : base + 100], sq4[:, sl], w2d[:, 60:80], start=True, stop=True)
                rsq34b = mid.tile([H, 2, 40], f32, name="rsq34b")  # [n3c | n4d] rsqrt per c
                nc.vector.reciprocal(
                    rsq34b,
                    psSmall[:, 60:260].rearrange("p (c x) -> p c x", c=2)[:, :, 0:40],
                )
                nc.scalar.sqrt(rsq34b, rsq34b)
                for c in range(2):
                    base = 100 * c
                    rsq34 = rsq34b[:, c, :]
                    sgn = sgn0 if c == 0 else sgn1
                    # t34 = [num3|num4] * [rs1c3|rs1c4] in one 40-col TT
                    t34 = mid.tile([H, 40], f32, name="t34")
                    nc.vector.tensor_tensor(
                        t34,
                        psSmall[:, base + 20 : base + 60],
                        rsa[:, 81 * c + 40 : 81 * c + 80],
                        op=OP.mult,
                    )
                    t3b = mid.tile([H, 20], f32, name="t3b")
                    nc.vector.tensor_tensor(t3b, t34[:, 0:20], rsq34[:, 0:20], op=OP.mult)
                    nc.scalar.mul(
                        outts[c][:, 80 + d * 20 : 80 + d * 20 + 20], t3b, sgn
                    )
                    nc.vector.tensor_tensor(
                        outts[c][:, 120 + d * 20 : 120 + d * 20 + 20], t34[:, 20:40], rsq34[:, 20:40], op=OP.mult
                    )

                # ---- m2: w2-scaled v2 (TSP 4x), then rep scale (TT 2x) +
                # matmul + paired PSUM drains ----
                v2wall = stg.tile([H, L, S], f16, name="v2wall")
                for l in range(L):
                    ts = nc.vector.tensor_scalar_mul(
                        v2wall[:, l, :], v2Tb, w2d[:, 20 + l : 21 + l]
                    )
                    if l == 0:
                        dep(ts, abs_v2Tb_dve, "v2Tb absorbed on DVE")
                stage = stg.tile([H, 2, L, S], f16, name="stage")
                tr128 = treep.tile([H, 2, L, 128], f16, name="tr128")
                tr64 = treep.tile([H, 2, L, 64], f16, name="tr64")
                for j in range(L // 2):
                    psNum = psNum_p.tile([H, 1024], f32, name="psNum")
                    l0 = 2 * j
                    v2ws = m2p.tile([H, 2, S], f16, name="v2ws")
                    nc.vector.tensor_tensor(
                        v2ws, v2wall[:, l0 : l0 + 2, :], repb[:, l0 : l0 + 2, :], op=OP.mult
                    )
                    for k in range(2):
                        n_mm0 = nc.tensor.matmul(psNum[:, 512 * k : 512 * k + 256], v1Tb[:, 0:H], v2ws[:, k, :], start=True, stop=True)
                        n_mm1 = nc.tensor.matmul(psNum[:, 512 * k + 256 : 512 * k + 512], v1Tb[:, H:S], v2ws[:, k, :], start=True, stop=True)
                        dep(n_mm0, ab3, "v1Tb absorbed before num2")
                        dep(n_mm1, ab3, "v1Tb absorbed before num2")
                    nc.scalar.copy(
                        stage[:, :, 2 * j : 2 * j + 2, :],
                        psNum[:].rearrange("p (j2 c q) -> p c j2 q", j2=2, c=2),
                    )
                    if j % 2 == 1:
                        # tree levels 1+2 on the 4 perspectives just drained:
                        # fills DVE slack inside the ACT-paced loop window
                        lo = 4 * (j // 2)
                        nc.vector.tensor_tensor(
                            tr128[:, :, lo : lo + 4, :],
                            stage[:, :, lo : lo + 4, 0:128],
                            stage[:, :, lo : lo + 4, 128:256],
                            op=OP.max,
                        )
                        nc.vector.tensor_tensor(
                            tr64[:, :, lo : lo + 4, :],
                            tr128[:, :, lo : lo + 4, 0:64],
                            tr128[:, :, lo : lo + 4, 64:128],
                            op=OP.max,
                        )
                pend = ((stage, rsa, outts, b, d), tr64)
        emit_m2_tail_rest(*pend)

    return nc


def _prep_core_inputs(p, q, w_list, core):
    """Host-side layout prep for one core. Only layout transforms + weight-only math."""
    sl = slice(core * BPC, (core + 1) * BPC)
    p8 = np.ascontiguousarray(p[sl])  # (BPC, 256, 256)
    q8 = np.ascontiguousarray(q[sl])
    # (BPC, 2, H, S): [b, d] = p8[b, :, d*H:(d+1)*H].T
    pT = np.ascontiguousarray(p8.reshape(BPC, S, 2, H).transpose(0, 2, 3, 1))
    qT = np.ascontiguousarray(q8.reshape(BPC, S, 2, H).transpose(0, 2, 3, 1))
    qn = np.ascontiguousarray(q8.reshape(BPC, S, 2, H).transpose(0, 2, 1, 3))
    q0n = np.ascontiguousarray(q[0].reshape(S, 2, H).transpose(1, 0, 2))  # (2, S, H)
    q0nb = q0n.astype(np.float16)

    w2T = np.empty((2, H, 81), np.float32)
    for d in range(2):
        ws = w_list[d::2]  # fw: w1,w3,w5,w7 ; bw: w2,w4,w6,w8
        cat = np.concatenate([w * w for w in ws] + [np.ones((1, H), np.float32)], 0)  # (81, H)
        w2T[d] = cat.T
    iota2 = np.stack([np.arange(H, dtype=np.float32), np.arange(H, 2 * H, dtype=np.float32)], 1)

    return {
        "pT": pT,
        "pTb": pT.astype(np.float16),
        "qT": qT,
        "qTb": qT.astype(np.float16),
        "qn": qn,
        "q0nb": q0nb,
        "w2T": w2T,
        "iota2": np.ascontiguousarray(iota2),
        "onesr": np.ones((1, H), np.float32),
        "ident": np.eye(H, dtype=np.float32),
    }


def _legalize_bir(bir_bytes):
    """This walrus build rejects >1 sync-wait command per instruction
    ("Too many sync wait commands"), while Drain accepts many. Move all but
    one wait of each instruction onto an inserted same-engine Drain."""
    import json as _json

    d = _json.loads(bir_bytes)
    n = 0
    for fnd in d["functions"]:
        for blk in fnd["blocks"]:
            insts = blk.get("instructions") or []
            out = []
            for ins in insts:
                si = ins.get("sync_info") or {}
                w = si.get("on_wait") or []
                if len(w) > 1:
                    for extra in w[:-1]:
                        out.append(
                            {
                                "debug": ins.get("debug", 0),
                                "engine": ins.get("engine"),
                                "ins": [],
                                "outs": [],
                                "is_reset_sema": False,
                                "name": f"I-legalw-{n}",
                                "opcode": "Drain",
                                "sync_info": {"on_update": [], "on_wait": [extra]},
                            }
                        )
                        n += 1
                    si["on_wait"] = [w[-1]]
                out.append(ins)
            blk["instructions"] = out
    return _json.dumps(d).encode(), n


def _install_legalizer():
    if _cache.get("legalizer"):
        return
    from concourse import bass2jax, bass_utils

    orig = bass_utils.compile_bir_kernel

    def patched(bir_json, tmpdir, neff_name="file.neff"):
        fixed, n = _legalize_bir(bir_json)
        return orig(fixed, tmpdir, neff_name)

    bass2jax.compile_bir_kernel = patched
    _cache["legalizer"] = True


def _get_runner():
    """Build the 8-core shard_map'd PJRT callable once (modeled on
    concourse.bass2jax.run_bass_via_pjrt, but cached so repeat calls skip
    trace + compile, and timing loops can reuse device-resident inputs)."""
    if "runner" in _cache:
        return _cache["runner"]

    import jax
    from jax.sharding import Mesh, PartitionSpec
    from jax.experimental.shard_map import shard_map

    import concourse.mybir as mybir
    from concourse import bass2jax

    if "nc" not in _cache:
        _cache["nc"] = _build_bass()
    nc = _cache["nc"]

    bass2jax.install_neuronx_cc_hook()
    _install_legalizer()
    assert nc.dbg_addr is None
    partition_name = nc.partition_id_tensor.name if nc.partition_id_tensor else None

    in_names, out_names, out_avals, zero_outs = [], [], [], []
    for alloc in nc.m.functions[0].allocations:
        if not isinstance(alloc, mybir.MemoryLocationSet):
            continue
        name = alloc.memorylocations[0].name
        if alloc.kind == "ExternalInput":
            if name != partition_name:
                in_names.append(name)
        elif alloc.kind == "ExternalOutput":
            out_names.append(name)
            shape = tuple(alloc.tensor_shape)
            dtype = mybir.dt.np(alloc.dtype)
            out_avals.append(jax.core.ShapedArray(shape, dtype))
            zero_outs.append(np.zeros(shape, dtype))
    n_params = len(in_names)
    n_outs = len(out_avals)
    all_names = in_names + out_names
    if partition_name is not None:
        all_names = all_names + [partition_name]

    def _body(*args):
        operands = list(args)
        if partition_name is not None:
            operands.append(bass2jax.partition_id_tensor())
        outs = bass2jax._bass_exec_p.bind(
            *operands,
            out_avals=tuple(out_avals),
            in_names=tuple(all_names),
            out_names=tuple(out_names),
            lowering_input_output_aliases=(),
            sim_require_finite=True,
            sim_require_nnan=True,
            nc=nc,
        )
        return tuple(outs)

    devices = jax.devices()[:NCORES]
    mesh = Mesh(np.asarray(devices), ("core",))
    sharded = jax.jit(
        shard_map(
            _body,
            mesh=mesh,
            in_specs=(PartitionSpec("core"),) * (n_params + n_outs),
            out_specs=(PartitionSpec("core"),) * n_outs,
            check_rep=False,
        ),
        donate_argnums=tuple(range(n_params, n_params + n_outs)),
        keep_unused=True,
    )
    runner = {
        "jax": jax,
        "sharded": sharded,
        "mesh": mesh,
        "in_names": in_names,
        "out_names": out_names,
        "out_avals": out_avals,
        "zero_outs": zero_outs,
        "n_params": n_params,
    }
    _cache["runner"] = runner
    return runner


def kernel(p, q, w1, w2, w3, w4, w5, w6, w7, w8, _time_iters=0):
    p = np.asarray(p, dtype=np.float32)
    q = np.asarray(q, dtype=np.float32)
    w_list = [np.asarray(w, dtype=np.float32) for w in (w1, w2, w3, w4, w5, w6, w7, w8)]

    r = _get_runner()
    jax = r["jax"]
    in_maps = [_prep_core_inputs(p, q, w_list, c) for c in range(NCORES)]
    concat_in = [
        np.concatenate([in_maps[c][name] for c in range(NCORES)], 0)
        for name in r["in_names"]
    ]
    concat_zeros = [
        np.zeros((NCORES * z.shape[0], *z.shape[1:]), z.dtype) for z in r["zero_outs"]
    ]
    out_arrs = r["sharded"](*concat_in, *concat_zeros)
    jax.block_until_ready(out_arrs)
    out = np.asarray(out_arrs[r["out_names"].index("out")])  # (64, 256, 160)

    if _time_iters:
        import time

        from jax.sharding import NamedSharding, PartitionSpec

        shd = NamedSharding(r["mesh"], PartitionSpec("core"))
        dev_in = [jax.device_put(a, shd) for a in concat_in]
        jax.block_until_ready(dev_in)
        times = []
        for _ in range(_time_iters):
            zeros = [
                jax.device_put(np.zeros((NCORES * z.shape[0], *z.shape[1:]), z.dtype), shd)
                for z in r["zero_outs"]
            ]
            jax.block_until_ready(zeros)
            t0 = time.perf_counter()
            o = r["sharded"](*dev_in, *zeros)
            jax.block_until_ready(o)
            times.append(time.perf_counter() - t0)
        kernel.last_exec_time_ns = int(min(times) * 1e9)
        kernel.all_times_ns = [int(t * 1e9) for t in times]
    return out



# revision 6
# speedup vs baseline: 193.2605x; 193.2605x over previous
"""BiMPM matching-layer kernel for Trainium2 (8 NeuronCores, pure data parallel).

Computes, per batch b and direction d (fw/bw, H=128 halves of the 256-dim inputs):
  m1: full matching vs last/first q timestep
  m2: max-pooling matching (max over q of per-perspective cosine)
  m3: mean-attentive matching (cosine-matrix weighted mean, scale-invariant form)
  m4: max-attentive matching (argmax over q of cosine matrix; gathers batch-0 q rows,
      faithful to the reference's flattened-index behavior)
Output (B, S1, 8*L) with L=20 perspectives.

Sharding: batch dim 64 -> 8 batches per core.  The 16 (b,d) pairs per core run
through a 3-deep software pipeline (emission order A(i), B(i-1), C(i-2)):
  A: input DMAs, squares, norm matmuls, rsqrt chains, rs2 DRAM-bounce broadcast,
     m1 numerators
  B: cosine-matrix paths (GT, mean-attentive GWT, argmax + index bounce), m1/m3
     sign prep
  C: m2 (w2-scaled v2 on DVE, rs2 scale on Pool, 1024-col fp16 matmuls, drains
     split ACT-copy/DVE-reduce_max-on-PSUM), m4 mask/gather path, finals, output
so that every bounce-DMA latency and cross-engine chain overlaps other pairs'
work and the PE never idles long enough to re-throttle (HAM).
"""
import numpy as np

B, S, H, L = 64, 256, 128, 20
NCORES = 8
BPC = B // NCORES
NP = 2 * BPC  # pairs per core
EPS = 1e-8

# m2 drain split: per chunk, l-blocks 0..ACT_J-1 drain via ACT into the f16
# stage (tree/reduce on DVE later); blocks ACT_J..4 reduce directly from PSUM
# on DVE. 4 l's per block.
ACT_J = 3

_cache = {}


def _build_bass():
    from contextlib import ExitStack

    import concourse.bass as bass
    import concourse.tile as tile
    from concourse import mybir

    f32 = mybir.dt.float32
    f16 = mybir.dt.float16
    bf16 = mybir.dt.bfloat16
    AF = mybir.ActivationFunctionType
    OP = mybir.AluOpType
    AX = mybir.AxisListType

    nc = bass.Bass()

    # DRAM I/O (per core)
    mrgF = nc.dram_tensor("mrgF", [BPC, H, 2, 768], f32, kind="ExternalInput")
    mrgH = nc.dram_tensor("mrgH", [BPC, H, 2, 512], f16, kind="ExternalInput")
    w2T = nc.dram_tensor("w2T", [2, H, 81], f32, kind="ExternalInput")
    q0nb = nc.dram_tensor("q0nb", [2, S, H], f16, kind="ExternalInput")
    iota2 = nc.dram_tensor("iota2", [H, 2], f32, kind="ExternalInput")
    onesr = nc.dram_tensor("onesr", [1, H], f32, kind="ExternalInput")
    ident = nc.dram_tensor("ident", [H, H], f32, kind="ExternalInput")
    out = nc.dram_tensor("out", [BPC, S, 8 * L], f32, kind="ExternalOutput")
    # DRAM bounces for row broadcasts (per-pair slots: no WAR hazards)
    r2scr = nc.dram_tensor("r2scr", [BPC, 2, 1, L * S], f16, kind="Internal")
    idxscr = nc.dram_tensor("idxscr", [BPC, 2, 1, S], f16, kind="Internal")

    with tile.TileContext(nc) as tc, ExitStack() as ctx:
        cons = ctx.enter_context(tc.tile_pool(name="cons", bufs=1))
        batchp = ctx.enter_context(tc.tile_pool(name="batchp", bufs=3))
        apool = ctx.enter_context(tc.tile_pool(name="apool", bufs=3))
        repp = ctx.enter_context(tc.tile_pool(name="repp", bufs=3))
        bpool = ctx.enter_context(tc.tile_pool(name="bpool", bufs=2))
        cpoolx = ctx.enter_context(tc.tile_pool(name="cpoolx", bufs=2))
        v2wsp = ctx.enter_context(tc.tile_pool(name="v2wsp", bufs=5))
        outp = ctx.enter_context(tc.tile_pool(name="outp", bufs=2))
        # PSUM: 8 banks = psNum 2x[H,1024](4) + psA 3x[H,512](3) + psB 1x[H,512](1)
        psNum_p = ctx.enter_context(tc.tile_pool(name="psNum", bufs=2, space="PSUM"))
        psA_p = ctx.enter_context(tc.tile_pool(name="psA", bufs=3, space="PSUM"))
        psB_p = ctx.enter_context(tc.tile_pool(name="psB", bufs=1, space="PSUM"))

        # ---- constants ----
        w2_sb = cons.tile([H, 2, 81], f32)
        nc.sync.dma_start(out=w2_sb, in_=w2T[:].rearrange("d h c -> h d c"))
        q0cb = cons.tile([H, 2, 2, H], f16)
        nc.sync.dma_start(out=q0cb, in_=q0nb[:].rearrange("d (c q) h -> q d c h", c=2))
        iota_sb = cons.tile([H, 2], f32)
        nc.sync.dma_start(out=iota_sb, in_=iota2[:])
        onesr_sb = cons.tile([1, H], f32)
        nc.sync.dma_start(out=onesr_sb, in_=onesr[:])
        ident_sb = cons.tile([H, H], f32)
        nc.sync.dma_start(out=ident_sb, in_=ident[:])
        # Touch every const once so const-DMA waits are absorbed here and later
        # instructions don't re-carry them.
        warm = cons.tile([H, 8], f32)
        nc.vector.tensor_copy(warm[:, 0:1], w2_sb[:, 0, 0:1])
        nc.vector.tensor_copy(warm[:, 2:3].bitcast(f16)[:, 0:1], q0cb[:, 0, 0, 0:1])
        nc.vector.tensor_copy(warm[:, 3:4], iota_sb[:, 0:1])
        nc.vector.tensor_copy(warm[0:1, 4:5], onesr_sb[0:1, 0:1])
        nc.vector.tensor_copy(warm[:, 5:6], ident_sb[:, 0:1])
        ab_w2 = nc.tensor.ldweights(w2_sb[:, 0, 0:1].bitcast(bf16))
        ab_q0 = nc.tensor.ldweights(q0cb[:, 0, 0, 0:1])
        ab_id = nc.tensor.ldweights(ident_sb[:, 0:1].bitcast(bf16))
        bass._add_dep_helper(ab_q0.ins, ab_w2.ins, sync=False, reason="const absorb chain")
        bass._add_dep_helper(ab_id.ins, ab_q0.ins, sync=False, reason="const absorb chain")

        def dep(from_inst, to_inst, why="absorb order"):
            bass._add_dep_helper(from_inst.ins, to_inst.ins, sync=False, reason=why)

        # per-pair state carried between phases
        st = [dict() for _ in range(NP)]
        batch_st = [dict() for _ in range(BPC)]
        last_pe_absorb = [ab_id]

        def emit_A(i):
            b, d = i // 2, i % 2
            first = i == 0
            if d == 0:
                bF = batchp.tile([H, 2, 768], f32, name="bF")
                nc.sync.dma_start(out=bF, in_=mrgF[b])
                bH = batchp.tile([H, 2, 512], f16, name="bH")
                nc.sync.dma_start(out=bH, in_=mrgH[b])
                outt0 = outp.tile([H, 8 * L], f32, name="outt0")
                outt1 = outp.tile([H, 8 * L], f32, name="outt1")
                batch_st[b] = dict(bF=bF, bH=bH, outts=(outt0, outt1))
                # absorb the batch-load DMA sems once on DVE and PE
                scr = apool.tile([H, 2], f32, name="dve_scr")
                nc.vector.tensor_copy(scr[:, 0:1], bF[:, 0, 0:1])
                nc.vector.tensor_copy(scr[:, 1:2].bitcast(f16)[:, 0:1], bH[:, 0, 0:1])
                abF = nc.tensor.ldweights(bF[:, 0, 0:1].bitcast(bf16))
                abH = nc.tensor.ldweights(bH[:, 0, 0:1])
                dep(abF, last_pe_absorb[0], "pe absorb chain")
                dep(abH, abF, "pe absorb chain")
                last_pe_absorb[0] = abH
            bs = batch_st[b]
            bF, bH = bs["bF"], bs["bH"]
            v1Tf = bF[:, d, 0:256]
            v2Tf = bF[:, d, 256:512]
            w2d = w2_sb[:, d, :]

            # squares of v1|v2 in one Pool op
            sq = apool.tile([H, 512], f32, name="sq")
            nc.gpsimd.tensor_tensor(sq, bF[:, d, 0:512], bF[:, d, 0:512], op=OP.mult)

            pa = psA_p.tile([H, 512], f32, name="pa")
            # [0:81 n1-c0 | 81:162 n1-c1 | 162:164 n2ones | n2bT at [0:20,164:420]
            #  | 420:440 n2a row | 440:460 n2a-rep | 460:500 num1 | 500:502 Gr
            #  | idxT at [0:1,164:420] after n2bT is consumed ]
            mms = [
                nc.tensor.matmul(pa[:, 0:81], sq[:, 0:H], w2d, start=True, stop=True),
                nc.tensor.matmul(pa[:, 81:162], sq[:, H:S], w2d, start=True, stop=True),
                nc.tensor.matmul(pa[:, 162:163], sq[:, 256:384], w2d[:, 80:81], start=True, stop=True),
                nc.tensor.matmul(pa[:, 163:164], sq[:, 384:512], w2d[:, 80:81], start=True, stop=True),
                nc.tensor.matmul(pa[0:20, 164:420], w2d[:, 20:40], sq[:, 256:512], start=True, stop=True),
            ]
            if first:
                for mm in mms:
                    dep(mm, ab_id, "consts before first mms")

            # rsqrt of the [H,164] norms: reciprocal (DVE) then sqrt (ACT)
            rsa = apool.tile([H, 164], f32, name="rsa")
            nc.vector.reciprocal(rsa, pa[:, 0:164])
            nc.scalar.sqrt(rsa, rsa)
            # rs2 for m2: [20,256] rsqrt -> f16 -> DRAM bounce broadcast
            r2bT = apool.tile([20, 256], f32, name="r2bT")
            nc.vector.reciprocal(r2bT, pa[0:20, 164:420])
            r2bTb = apool.tile([20, 256], f16, name="r2bTb")
            nc.scalar.sqrt(r2bTb, r2bT)
            nc.sync.dma_start(
                out=r2scr[b, d].rearrange("o (l s) -> (o l) s", l=L), in_=r2bTb[:]
            )
            repb = repp.tile([H, L, S], f16, name="repb")
            nc.sync.dma_start(
                out=repb[:].rearrange("h l s -> h (l s)"),
                in_=r2scr[b, d].to_broadcast((H, L * S)),
            )

            # eps * ||v1|| per chunk (for the m3 sign)
            nv1e = apool.tile([H, 2], f32, name="nv1e")
            nc.scalar.sqrt(nv1e[:, 0:1], pa[:, 80:81])
            nc.scalar.mul(nv1e[:, 0:1], nv1e[:, 0:1], EPS)
            nc.scalar.sqrt(nv1e[:, 1:2], pa[:, 161:162])
            nc.scalar.mul(nv1e[:, 1:2], nv1e[:, 1:2], EPS)

            # m1 numerators
            tcol = v2Tf[:, 255:256] if d == 0 else v2Tf[:, 0:1]
            sqt = apool.tile([H, 1], f32, name="sqt")
            nc.vector.tensor_tensor(sqt, tcol, tcol, op=OP.mult)
            rhs1 = apool.tile([H, 20], f32, name="rhs1")
            nc.vector.tensor_scalar_mul(rhs1, w2d[:, 0:20], tcol)
            n2a_mm = nc.tensor.matmul(pa[0:1, 420:440], sqt, w2d[:, 0:20], start=True, stop=True)
            n2a_sb = apool.tile([1, 20], f32, name="n2a_sb")
            nc.vector.tensor_copy(n2a_sb, pa[0:1, 420:440])
            nc.tensor.matmul(pa[:, 440:460], onesr_sb, n2a_sb, start=True, stop=True)
            nc.tensor.matmul(pa[:, 460:480], v1Tf[:, 0:H], rhs1, start=True, stop=True)
            nc.tensor.matmul(pa[:, 480:500], v1Tf[:, H:S], rhs1, start=True, stop=True)
            rs2rep = apool.tile([H, 20], f32, name="rs2rep")
            nc.vector.reciprocal(rs2rep, pa[:, 440:460])
            nc.scalar.sqrt(rs2rep, rs2rep)

            st[i].update(
                pa=pa, rsa=rsa, repb=repb, rs2rep=rs2rep, nv1e=nv1e,
                v1Tf=v1Tf, v2Tf=v2Tf,
                v1Tb=bH[:, d, 0:256], v2Tb=bH[:, d, 256:512],
                v2nat=bF[:, d, 512:768].rearrange("p (c h) -> p c h", c=2),
                outts=bs["outts"], b=b, d=d,
            )

        def emit_B(i):
            s = st[i]
            b, d = s["b"], s["d"]
            pa, rsa = s["pa"], s["rsa"]
            v1Tf, v2Tf, v2nat = s["v1Tf"], s["v2Tf"], s["v2nat"]
            outts = s["outts"]

            pb = psB_p.tile([H, 512], f32, name="pb")
            # GT[q, (c p)] = v2^T v1
            nc.tensor.matmul(pb[:, 0:256], v2Tf[:, 0:H], v1Tf, start=True, stop=True)
            nc.tensor.matmul(pb[:, 256:512], v2Tf[:, H:S], v1Tf, start=True, stop=True)
            GT_sb = bpool.tile([H, 2, S], f32, name="GT_sb")
            nc.scalar.copy(GT_sb[:].rearrange("h c q -> h (c q)"), pb[:, 0:512])

            # v2r = v2 rows * rs2 (per-q-partition scale on ACT)
            v2r = bpool.tile([H, 2, H], f32, name="v2r")
            nc.scalar.activation(v2r[:, 0, :], v2nat[:, 0, :], AF.Copy, scale=rsa[:, 162:163])
            nc.scalar.activation(v2r[:, 1, :], v2nat[:, 1, :], AF.Copy, scale=rsa[:, 163:164])

            # GWT (mean-attentive, scale-invariant) at pb[0:256]; v2r^T at [256:512]
            nc.tensor.matmul(pb[:, 0:256], v2r[:, 0, :], GT_sb[:, 0, :], start=True, stop=False)
            nc.tensor.matmul(pb[:, 0:256], v2r[:, 1, :], GT_sb[:, 1, :], start=False, stop=True)
            tr0 = nc.tensor.transpose(pb[:, 256:384], v2r[:, 0, :], ident_sb)
            tr1 = nc.tensor.transpose(pb[:, 384:512], v2r[:, 1, :], ident_sb)
            if i == 0:
                dep(tr0, ab_id, "ident absorbed before transpose")
                dep(tr1, ab_id, "ident absorbed before transpose")
            v2nT = bpool.tile([H, S], f32, name="v2nT")
            nc.scalar.copy(v2nT, pb[:, 256:512])
            prod3 = bpool.tile([H, S], f32, name="prod3")
            nc.vector.tensor_tensor(prod3, v1Tf, pb[:, 0:256], op=OP.mult)
            sq3 = bpool.tile([H, S], f32, name="sq3")
            nc.scalar.square(sq3, pb[:, 0:256])

            # G' = v1 . v2n for argmax (rs1[p] scale drops out)
            nc.tensor.matmul(pb[:, 0:256], v1Tf[:, 0:H], v2nT, start=True, stop=True)
            nc.tensor.matmul(pb[:, 256:512], v1Tf[:, H:S], v2nT, start=True, stop=True)
            idxf = bpool.tile([H, 2], f32, name="idxf")
            for c in range(2):
                top8 = bpool.tile([H, 8], f32, name="top8")
                idx8 = bpool.tile([H, 8], mybir.dt.uint32, name="idx8")
                nc.vector.max_with_indices(top8, idx8, pb[:, 256 * c : 256 * c + 256])
                nc.vector.tensor_copy(idxf[:, c : c + 1], idx8[:, 0:1])
            for c in range(2):
                tr = nc.tensor.transpose(
                    pa[0:1, 164 + c * H : 164 + c * H + H], idxf[:, c : c + 1], ident_sb
                )
                if i == 0:
                    dep(tr, ab_id, "ident absorbed before transpose")
            idxT_sb = bpool.tile([1, 256], f16, name="idxT_sb")
            nc.scalar.copy(idxT_sb, pa[0:1, 164:420])
            nc.sync.dma_start(out=idxscr[b, d], in_=idxT_sb[:])
            idxrepb = bpool.tile([H, 256], f16, name="idxrepb")
            nc.sync.dma_start(out=idxrepb, in_=idxscr[b, d].to_broadcast((H, S)))

            # m3 sign: Gr = sum_q num[q,p]*rs2[q], sgn = Sign(Gr + eps||v1||)
            for c in range(2):
                nc.tensor.matmul(pa[:, 500 + c : 501 + c], GT_sb[:, 0, c * H : c * H + H],
                                 rsa[:, 162:163], start=True, stop=False)
                nc.tensor.matmul(pa[:, 500 + c : 501 + c], GT_sb[:, 1, c * H : c * H + H],
                                 rsa[:, 163:164], start=False, stop=True)
            sgn = bpool.tile([H, 2], f32, name="sgn")
            nc.scalar.activation(sgn[:, 0:1], pa[:, 500:501], AF.Sign, bias=s["nv1e"][:, 0:1], scale=1.0)
            nc.scalar.activation(sgn[:, 1:2], pa[:, 501:502], AF.Sign, bias=s["nv1e"][:, 1:2], scale=1.0)

            # m1 finals
            t1b = bpool.tile([H, 2, 20], f32, name="t1b")
            nc.vector.tensor_tensor(
                t1b,
                pa[:, 460:500].rearrange("p (c x) -> p c x", c=2),
                rsa[:, 0:162].rearrange("p (c x) -> p c x", c=2)[:, :, 0:20],
                op=OP.mult,
            )
            for c in range(2):
                nc.vector.tensor_tensor(
                    outts[c][:, d * 20 : d * 20 + 20], t1b[:, c, :], s["rs2rep"], op=OP.mult
                )
            s.update(GT_sb=GT_sb, prod3=prod3, sq3=sq3, idxrepb=idxrepb, sgn=sgn)

        def emit_C(i):
            s = st[i]
            b, d = s["b"], s["d"]
            pa, rsa, repb = s["pa"], s["rsa"], s["repb"]
            v1Tf, v1Tb, v2Tb = s["v1Tf"], s["v1Tb"], s["v2Tb"]
            outts = s["outts"]
            w2d = w2_sb[:, d, :]

            # masks from the (landed) idx broadcast, then att4 gather + m3/m4 nums
            maskT0 = cpoolx.tile([H, 256], f16, name="maskT0")
            nc.vector.tensor_scalar(maskT0, s["idxrepb"], iota_sb[:, 0:1], None, op0=OP.is_equal)
            maskT1 = cpoolx.tile([H, 256], f16, name="maskT1")
            nc.vector.tensor_scalar(maskT1, s["idxrepb"], iota_sb[:, 1:2], None, op0=OP.is_equal)

            pc0 = psNum_p.tile([H, 4, 256], f32, name="psNum")
            pc0f = pc0[:].rearrange("p l q -> p (l q)")
            a4_mm0 = nc.tensor.matmul(pc0f[:, 160:416], q0cb[:, d, 0, :], maskT0, start=True, stop=False)
            a4_mm1 = nc.tensor.matmul(pc0f[:, 160:416], q0cb[:, d, 1, :], maskT1, start=False, stop=True)
            if i == 0:
                dep(a4_mm0, ab_id, "q0c absorbed before att4T")
                dep(a4_mm1, ab_id, "q0c absorbed before att4T")
            # m3 nums: [0:20 num3-c0 | 20:40 num3-c1 | 40:80 (unused) |
            #           80:100 n3-c0 | 100:120 n3-c1 ... wait see layout below]
            # layout: [0:40 num3 (c0,c1) | 40:80 num4 (c0,c1) | 80:120 n3 | 120:160 n4]
            prod3, sq3 = s["prod3"], s["sq3"]
            for c in range(2):
                sl = slice(c * H, c * H + H)
                nc.tensor.matmul(pc0f[:, c * 20 : c * 20 + 20], prod3[:, sl], w2d[:, 40:60], start=True, stop=True)
                nc.tensor.matmul(pc0f[:, 80 + c * 20 : 80 + c * 20 + 20], sq3[:, sl], w2d[:, 40:60], start=True, stop=True)
            prod4 = cpoolx.tile([H, S], f32, name="prod4")
            nc.vector.tensor_tensor(prod4, v1Tf, pc0f[:, 160:416], op=OP.mult)
            sq4 = cpoolx.tile([H, S], f32, name="sq4")
            nc.scalar.square(sq4, pc0f[:, 160:416])
            for c in range(2):
                sl = slice(c * H, c * H + H)
                nc.tensor.matmul(pc0f[:, 40 + c * 20 : 40 + c * 20 + 20], prod4[:, sl], w2d[:, 60:80], start=True, stop=True)
                nc.tensor.matmul(pc0f[:, 120 + c * 20 : 120 + c * 20 + 20], sq4[:, sl], w2d[:, 60:80], start=True, stop=True)

            # m3/m4 finals
            rsq34 = cpoolx.tile([H, 80], f32, name="rsq34")
            nc.vector.reciprocal(rsq34, pc0f[:, 80:160])
            nc.scalar.sqrt(rsq34, rsq34)
            t34 = cpoolx.tile([H, 2, 2, 20], f32, name="t34")  # [m34, c, l]
            for j in range(2):  # 0: m3 (w5/w6 rs1), 1: m4 (w7/w8 rs1)
                nc.vector.tensor_tensor(
                    t34[:, j],
                    pc0f[:, j * 40 : j * 40 + 40].rearrange("p (c x) -> p c x", c=2),
                    rsa[:, 0:162].rearrange("p (c x) -> p c x", c=2)[:, :, 40 + 20 * j : 60 + 20 * j],
                    op=OP.mult,
                )
            t34b = cpoolx.tile([H, 2, 2, 20], f32, name="t34b")
            nc.vector.tensor_tensor(
                t34b, t34,
                rsq34[:].rearrange("p (j c x) -> p j c x", j=2, c=2),
                op=OP.mult,
            )
            for c in range(2):
                nc.scalar.mul(
                    outts[c][:, 80 + d * 20 : 80 + d * 20 + 20], t34b[:, 0, c, :], s["sgn"][:, c : c + 1]
                )
                nc.vector.tensor_copy(
                    outts[c][:, 120 + d * 20 : 120 + d * 20 + 20], t34b[:, 1, c, :]
                )

            # ---- m2 ----
            v2wall = cpoolx.tile([H, L, S], f16, name="v2wall")
            for l in range(L):
                nc.vector.tensor_scalar_mul(v2wall[:, l, :], v2Tb, w2d[:, 20 + l : 21 + l])
            v2ws = []
            for j in range(5):
                t = v2wsp.tile([H, 4, S], f16, name=f"v2ws{j}")
                nc.gpsimd.tensor_tensor(
                    t, v2wall[:, 4 * j : 4 * j + 4, :], repb[:, 4 * j : 4 * j + 4, :], op=OP.mult
                )
                v2ws.append(t)

            stage = cpoolx.tile([H, 2, 4 * ACT_J, S], f16, name="stage")
            m2pre = cpoolx.tile([H, 2, L], f32, name="m2pre")
            for c in range(2):
                for j in range(5):
                    pc = psNum_p.tile([H, 4, 256], f32, name="psNum")
                    pcf = pc[:].rearrange("p l q -> p (l q)")
                    vwf = v2ws[j][:].rearrange("p l q -> p (l q)")
                    mm = nc.tensor.matmul(
                        pcf[:, 0:512], v1Tb[:, c * H : c * H + H], vwf[:, 0:512],
                        start=True, stop=True,
                    )
                    nc.tensor.matmul(
                        pcf[:, 512:1024], v1Tb[:, c * H : c * H + H], vwf[:, 512:1024],
                        start=True, stop=True,
                    )
                    if i == 0 and c == 0 and j == 0:
                        dep(mm, last_pe_absorb[0], "absorbs before m2")
                    if j < ACT_J:
                        nc.scalar.copy(stage[:, c, 4 * j : 4 * j + 4, :], pc[:])
                    else:
                        nc.vector.tensor_reduce(
                            m2pre[:, c, 4 * j : 4 * j + 4], pc[:], axis=AX.X, op=OP.max
                        )
                # reduce the ACT-drained l's from the f16 stage
                nc.vector.tensor_reduce(
                    m2pre[:, c, 0 : 4 * ACT_J], stage[:, c], axis=AX.X, op=OP.max
                )
            for c in range(2):
                nc.vector.tensor_tensor(
                    outts[c][:, 40 + d * 20 : 40 + d * 20 + 20],
                    m2pre[:, c, :],
                    rsa[:, 81 * c + 20 : 81 * c + 40],
                    op=OP.mult,
                )
            if d == 1:
                nc.sync.dma_start(out=out[b, 0:H, :], in_=outts[0])
                nc.sync.dma_start(out=out[b, H:S, :], in_=outts[1])

        for r in range(NP + 2):
            if r < NP:
                emit_A(r)
            if 1 <= r <= NP:
                emit_B(r - 1)
            if r >= 2:
                emit_C(r - 2)

    return nc


def _prep_core_inputs(p, q, w_list, core):
    """Host-side layout prep for one core. Only layout transforms + weight-only math."""
    sl = slice(core * BPC, (core + 1) * BPC)
    p8 = np.ascontiguousarray(p[sl])  # (BPC, 256, 256)
    q8 = np.ascontiguousarray(q[sl])
    # [b, h, d, 0:256]=pT, [256:512]=qT, [512:768]=qn rows (c,h)
    pT = p8.reshape(BPC, S, 2, H).transpose(0, 3, 2, 1)  # (BPC, H, 2, S)
    qT = q8.reshape(BPC, S, 2, H).transpose(0, 3, 2, 1)
    # qn[b, qp, d, c, h] = q8[b, c*128+qp, d*H+h]
    qn = q8.reshape(BPC, 2, H, 2, H).transpose(0, 2, 3, 1, 4)  # (BPC, qp, d, c, h)
    mrgF = np.empty((BPC, H, 2, 768), np.float32)
    mrgF[..., 0:256] = pT
    mrgF[..., 256:512] = qT
    mrgF[..., 512:768] = qn.reshape(BPC, H, 2, 256)
    mrgH = np.empty((BPC, H, 2, 512), np.float16)
    mrgH[..., 0:256] = pT
    mrgH[..., 256:512] = qT

    q0n = np.ascontiguousarray(q[0].reshape(S, 2, H).transpose(1, 0, 2))  # (2, S, H)

    w2T = np.empty((2, H, 81), np.float32)
    for d in range(2):
        ws = w_list[d::2]  # fw: w1,w3,w5,w7 ; bw: w2,w4,w6,w8
        cat = np.concatenate([w * w for w in ws] + [np.ones((1, H), np.float32)], 0)
        w2T[d] = cat.T
    iota2 = np.stack([np.arange(H, dtype=np.float32), np.arange(H, 2 * H, dtype=np.float32)], 1)

    return {
        "mrgF": mrgF,
        "mrgH": mrgH,
        "w2T": w2T,
        "q0nb": q0n.astype(np.float16),
        "iota2": np.ascontiguousarray(iota2),
        "onesr": np.ones((1, H), np.float32),
        "ident": np.eye(H, dtype=np.float32),
    }


def _legalize_bir(bir_bytes):
    """This walrus build rejects >1 sync-wait command per instruction; move all
    but one wait of each instruction onto an inserted same-engine Drain."""
    import json as _json

    d = _json.loads(bir_bytes)
    n = 0
    for fnd in d["functions"]:
        for blk in fnd["blocks"]:
            insts = blk.get("instructions") or []
            out = []
            for ins in insts:
                si = ins.get("sync_info") or {}
                w = si.get("on_wait") or []
                if len(w) > 1:
                    for extra in w[:-1]:
                        out.append(
                            {
                                "debug": ins.get("debug", 0),
                                "engine": ins.get("engine"),
                                "ins": [],
                                "outs": [],
                                "is_reset_sema": False,
                                "name": f"I-legalw-{n}",
                                "opcode": "Drain",
                                "sync_info": {"on_update": [], "on_wait": [extra]},
                            }
                        )
                        n += 1
                    si["on_wait"] = [w[-1]]
                out.append(ins)
            blk["instructions"] = out
    return _json.dumps(d).encode(), n


def _install_legalizer():
    if _cache.get("legalizer"):
        return
    from concourse import bass2jax, bass_utils

    orig = bass_utils.compile_bir_kernel

    def patched(bir_json, tmpdir, neff_name="file.neff"):
        fixed, n = _legalize_bir(bir_json)
        return orig(fixed, tmpdir, neff_name)

    bass2jax.compile_bir_kernel = patched
    _cache["legalizer"] = True


def _get_runner():
    """Build the 8-core shard_map'd PJRT callable once."""
    if "runner" in _cache:
        return _cache["runner"]

    import jax
    from jax.sharding import Mesh, PartitionSpec
    from jax.experimental.shard_map import shard_map

    import concourse.mybir as mybir
    from concourse import bass2jax

    if "nc" not in _cache:
        _cache["nc"] = _build_bass()
    nc = _cache["nc"]

    bass2jax.install_neuronx_cc_hook()
    _install_legalizer()
    assert nc.dbg_addr is None
    partition_name = nc.partition_id_tensor.name if nc.partition_id_tensor else None

    in_names, out_names, out_avals, zero_outs = [], [], [], []
    for alloc in nc.m.functions[0].allocations:
        if not isinstance(alloc, mybir.MemoryLocationSet):
            continue
        name = alloc.memorylocations[0].name
        if alloc.kind == "ExternalInput":
            if name != partition_name:
                in_names.append(name)
        elif alloc.kind == "ExternalOutput":
            out_names.append(name)
            shape = tuple(alloc.tensor_shape)
            dtype = mybir.dt.np(alloc.dtype)
            out_avals.append(jax.core.ShapedArray(shape, dtype))
            zero_outs.append(np.zeros(shape, dtype))
    n_params = len(in_names)
    n_outs = len(out_avals)
    all_names = in_names + out_names
    if partition_name is not None:
        all_names = all_names + [partition_name]

    def _body(*args):
        operands = list(args)
        if partition_name is not None:
            operands.append(bass2jax.partition_id_tensor())
        outs = bass2jax._bass_exec_p.bind(
            *operands,
            out_avals=tuple(out_avals),
            in_names=tuple(all_names),
            out_names=tuple(out_names),
            lowering_input_output_aliases=(),
            sim_require_finite=True,
            sim_require_nnan=True,
            nc=nc,
        )
        return tuple(outs)

    devices = jax.devices()[:NCORES]
    mesh = Mesh(np.asarray(devices), ("core",))
    sharded = jax.jit(
        shard_map(
            _body,
            mesh=mesh,
            in_specs=(PartitionSpec("core"),) * (n_params + n_outs),
            out_specs=(PartitionSpec("core"),) * n_outs,
            check_rep=False,
        ),
        donate_argnums=tuple(range(n_params, n_params + n_outs)),
        keep_unused=True,
    )
    runner = {
        "jax": jax,
        "sharded": sharded,
        "mesh": mesh,
        "in_names": in_names,
        "out_names": out_names,
        "out_avals": out_avals,
        "zero_outs": zero_outs,
        "n_params": n_params,
    }
    _cache["runner"] = runner
    return runner


def kernel(p, q, w1, w2, w3, w4, w5, w6, w7, w8, _time_iters=0):
    p = np.asarray(p, dtype=np.float32)
    q = np.asarray(q, dtype=np.float32)
    w_list = [np.asarray(w, dtype=np.float32) for w in (w1, w2, w3, w4, w5, w6, w7, w8)]

    r = _get_runner()
    jax = r["jax"]
    in_maps = [_prep_core_inputs(p, q, w_list, c) for c in range(NCORES)]
    concat_in = [
        np.concatenate([in_maps[c][name] for c in range(NCORES)], 0)
        for name in r["in_names"]
    ]
    concat_zeros = [
        np.zeros((NCORES * z.shape[0], *z.shape[1:]), z.dtype) for z in r["zero_outs"]
    ]
    out_arrs = r["sharded"](*concat_in, *concat_zeros)
    jax.block_until_ready(out_arrs)
    out = np.asarray(out_arrs[r["out_names"].index("out")])  # (64, 256, 160)

    if _time_iters:
        import time

        from jax.sharding import NamedSharding, PartitionSpec

        shd = NamedSharding(r["mesh"], PartitionSpec("core"))
        dev_in = [jax.device_put(a, shd) for a in concat_in]
        jax.block_until_ready(dev_in)
        times = []
        for _ in range(_time_iters):
            zeros = [
                jax.device_put(np.zeros((NCORES * z.shape[0], *z.shape[1:]), z.dtype), shd)
                for z in r["zero_outs"]
            ]
            jax.block_until_ready(zeros)
            t0 = time.perf_counter()
            o = r["sharded"](*dev_in, *zeros)
            jax.block_until_ready(o)
            times.append(time.perf_counter() - t0)
        kernel.last_exec_time_ns = int(min(times) * 1e9)
        kernel.all_times_ns = [int(t * 1e9) for t in times]
    return out
